# revision 5
# baseline (speedup 1.0000x reference)
"""Trainium2 Bass kernel for a classical LSTM (T=512, B=64, D=H=1024).

Strategy (8 NeuronCores, one chip):
  - Tensor-parallel over the 4H gate dimension: core c owns a 128-column
    slice of each gate (f,i,g,o) -> a [D+H, 512] weight slice, and the
    matching H-slice of the c/h state.
  - Per step: gates_c[64,512] = xp_c[t] + h_{t-1} @ W_h_c  (8 accumulating
    PE matmuls with h^T stationary, weights moving), epilogue on ACT/DVE,
    PE-transpose of the new h slice [64,128] -> [128,64], AllGather over
    the 8 cores rebuilds the full h^T [1024,64] for the next step.
  - The input projection xp[t] = x_t @ W_x_c + b_c for all 512 steps is
    computed just-in-time ~16 steps ahead of the recurrence, on the same
    cores, hidden in the PE idle time while the AllGather is in flight.
  - X is pre-transposed on the host (layout marshalling), so phase 1 needs
    no on-device transposes.

Host <-> device contract (per core c):
  inputs:  xT   [1024, 32768]  f32   (= inputs.reshape(T*B, D).T, replicated)
           w    [2048, 512]    f32   (columns [g*1024 + c*128 : ... +128] of
                                      W_all = concat(W_f,W_i,W_g,W_o, axis=1),
                                      g = 0..3; rows = [D; H])
           bias [128, 512]     f32   (matching bias slice, broadcast to rows)
  outputs: stacked_s [512, 64, 128] f32  (H-slice c of every h_t)
           cx_s      [64, 128]      f32  (H-slice c of final c)
"""
import numpy as np

N_CORES = 8
T, B, D, H = 512, 64, 1024, 1024
HS = H // N_CORES          # 128  per-core H slice
GS = 4 * HS                # 512  per-core gate-slice width
KD = D // 128              # 8    k-tiles in D
KH = H // 128              # 8    k-tiles in H
LOOK = 16                  # phase-1 lookahead in steps (must be even)

_BUILT = {}


def build(t_steps=T):
    """Build + schedule the bass program. Returns (nc, runner_builder)."""
    import concourse.bass as bass
    import concourse.mybir as mybir
    import concourse.tile as tile
    from concourse import bacc
    from concourse.masks import make_identity

    f32 = mybir.dt.float32

    nc = bacc.Bacc("TRN2", target_bir_lowering=False, debug=False,
                   num_devices=N_CORES)
    xT = nc.dram_tensor("xT", [D, t_steps * B], f32, kind="ExternalInput").ap()
    w = nc.dram_tensor("w", [D + H, GS], f32, kind="ExternalInput").ap()
    bias = nc.dram_tensor("bias", [128, GS], f32, kind="ExternalInput").ap()
    out_st = nc.dram_tensor("stacked_s", [t_steps, B, HS], f32,
                            kind="ExternalOutput").ap()
    out_c = nc.dram_tensor("cx_s", [B, HS], f32, kind="ExternalOutput").ap()

    xT_r = xT.rearrange("(k p) m -> p k m", p=128)   # [128, KD, T*B]
    w_r = w.rearrange("(k p) g -> p k g", p=128)     # [128, KD+KH, GS]

    with tile.TileContext(nc) as tc:
        with (
            tc.tile_pool(name="const", bufs=1) as constp,
            tc.tile_pool(name="state", bufs=1) as statep,
            tc.tile_pool(name="xp_ring", bufs=LOOK + 6) as xpp,
            tc.tile_pool(name="xt_in", bufs=3) as xtp,
            tc.tile_pool(name="hT", bufs=2) as hTp,
            tc.tile_pool(name="work", bufs=3) as workp,
            tc.tile_pool(name="ps_gates", bufs=2, space="PSUM") as psg,
            tc.tile_pool(name="ps_xp", bufs=2, space="PSUM") as psx,
            tc.tile_pool(name="ps_tr", bufs=2, space="PSUM") as pst,
            tc.tile_pool(name="dram", bufs=2, space="DRAM") as dramp,
        ):
            # ---- prologue: constants ----
            w_sb = constp.tile([128, KD + KH, GS], f32, name="w_sb")
            for k in range(KD + KH):
                nc.sync.dma_start(w_sb[:, k, :], w_r[:, k, :])
            bias_sb = constp.tile([128, GS], f32, name="bias_sb")
            nc.sync.dma_start(bias_sb[:], bias[:])
            ident = constp.tile([128, 128], f32, name="ident")
            make_identity(nc, ident[:])
            c_sb = statep.tile([B, HS], f32, name="c_sb")

            xp_tiles = {}

            def phase1_chunk(t0):
                """Compute xp for steps t0, t0+1 (t0 even)."""
                xt_sb = xtp.tile([128, KD, 128], f32, name="xt_sb")
                nc.sync.dma_start(
                    xt_sb[:], xT_r[:, :, t0 * B:(t0 + 2) * B]
                )
                ps = psx.tile([128, GS], f32, name="ps")
                for k in range(KD):
                    nc.tensor.matmul(
                        ps[:], xt_sb[:, k, :], w_sb[:, k, :],
                        start=(k == 0), stop=(k == KD - 1),
                    )
                for j in range(2):
                    xp_t = xpp.tile([B, GS], f32, name="xp_t",
                                    tag="xp_ring")
                    nc.vector.tensor_add(
                        xp_t[:], ps[j * B:(j + 1) * B, :], bias_sb[0:B, :]
                    )
                    xp_tiles[t0 + j] = xp_t

            n_pre = min(LOOK, t_steps)
            for t0 in range(0, n_pre, 2):
                phase1_chunk(t0)

            hT_prev = None
            for t in range(t_steps):
                if t % 2 == 0 and t + LOOK < t_steps:
                    phase1_chunk(t + LOOK)

                xp_t = xp_tiles.pop(t)
                if t == 0:
                    gates = xp_t  # h_{-1} = 0: gates are just xp
                else:
                    psg_t = psg.tile([B, GS], f32, name="psg_t")
                    for k in range(KH):
                        nc.tensor.matmul(
                            psg_t[:],
                            hT_prev[:, k, :],
                            w_sb[:, KD + k, :],
                            start=(k == 0), stop=(k == KH - 1),
                        )
                    gates = workp.tile([B, GS], f32, name="gates", tag="gates")
                    nc.vector.tensor_add(gates[:], psg_t[:], xp_t[:])

                # activations in place: [f | i | g | o] each HS wide
                _mb = mybir
                nc.scalar.activation(gates[:, 0:2 * HS], gates[:, 0:2 * HS],
                                     _mb.ActivationFunctionType.Sigmoid)
                nc.scalar.activation(gates[:, 2 * HS:3 * HS],
                                     gates[:, 2 * HS:3 * HS],
                                     _mb.ActivationFunctionType.Tanh)
                nc.scalar.activation(gates[:, 3 * HS:4 * HS],
                                     gates[:, 3 * HS:4 * HS],
                                     _mb.ActivationFunctionType.Sigmoid)

                f_g = gates[:, 0:HS]
                i_g = gates[:, HS:2 * HS]
                g_g = gates[:, 2 * HS:3 * HS]
                o_g = gates[:, 3 * HS:4 * HS]

                ig = workp.tile([B, HS], f32, name="ig", tag="ig")
                nc.vector.tensor_mul(ig[:], i_g, g_g)
                if t == 0:
                    nc.vector.tensor_copy(c_sb[:], ig[:])
                else:
                    fc = workp.tile([B, HS], f32, name="fc", tag="fc")
                    nc.vector.tensor_mul(fc[:], f_g, c_sb[:])
                    nc.vector.tensor_add(c_sb[:], fc[:], ig[:])

                th = workp.tile([B, HS], f32, name="th", tag="th")
                nc.scalar.activation(th[:], c_sb[:],
                                     _mb.ActivationFunctionType.Tanh)
                h_sb = workp.tile([B, HS], f32, name="h_sb", tag="h")
                nc.vector.tensor_mul(h_sb[:], o_g, th[:])

                nc.sync.dma_start(out_st[t, :, :], h_sb[:])

                if t == t_steps - 1:
                    nc.sync.dma_start(out_c[:], c_sb[:])
                    break

                # transpose h slice -> [128, 64] and AllGather full h^T
                tr = pst.tile([HS, B], f32, name="tr")
                nc.tensor.transpose(tr[:], h_sb[:], ident[0:B, 0:B])
                tr_sb = workp.tile([HS, B], f32, name="tr_sb", tag="tr_sb")
                nc.vector.tensor_copy(tr_sb[:], tr[:])
                ag_in = dramp.tile([HS, B], f32, name="ag_in", tag="ag_in")
                nc.sync.dma_start(ag_in[:], tr_sb[:])
                ag_out = dramp.tile([H, B], f32, name="ag_out", tag="ag_out")
                nc.gpsimd.collective_compute(
                    "AllGather",
                    _mb.AluOpType.bypass,
                    ins=[ag_in.opt()],
                    outs=[ag_out.opt()],
                    replica_groups=[list(range(N_CORES))],
                )
                hT = hTp.tile([128, KH, B], f32, name="hT", tag="hT")
                for k in range(KH):
                    nc.sync.dma_start(hT[:, k, :], ag_out[k * 128:(k + 1) * 128, :])
                hT_prev = hT

    nc.compile()
    return nc


class _Runner:
    """Reusable jitted SPMD runner (one trace/compile, many calls)."""

    def __init__(self, nc, n_cores):
        import jax
        import concourse.mybir as mybir
        from jax.sharding import Mesh, PartitionSpec
        from jax.experimental.shard_map import shard_map
        from concourse import bass2jax
        from concourse.bass2jax import _bass_exec_p, install_neuronx_cc_hook

        install_neuronx_cc_hook()
        self.jax = jax
        self.nc = nc
        self.n_cores = n_cores
        partition_name = (
            nc.partition_id_tensor.name if nc.partition_id_tensor else None
        )
        in_names, out_names, out_avals = [], [], []
        for alloc in nc.m.functions[0].allocations:
            if not isinstance(alloc, mybir.MemoryLocationSet):
                continue
            name = alloc.memorylocations[0].name
            if alloc.kind == "ExternalInput":
                if name != partition_name:
                    in_names.append(name)
            elif alloc.kind == "ExternalOutput":
                out_names.append(name)
                out_avals.append(
                    jax.core.ShapedArray(
                        tuple(alloc.tensor_shape), mybir.dt.np(alloc.dtype)
                    )
                )
        self.in_names, self.out_names, self.out_avals = (
            in_names, out_names, out_avals,
        )
        n_params = len(in_names)
        all_in = list(in_names) + list(out_names)
        if partition_name is not None:
            all_in.append(partition_name)

        def _body(*args):
            operands = list(args)
            if partition_name is not None:
                operands.append(bass2jax.partition_id_tensor())
            return tuple(
                _bass_exec_p.bind(
                    *operands,
                    out_avals=tuple(out_avals),
                    in_names=tuple(all_in),
                    out_names=tuple(out_names),
                    lowering_input_output_aliases=(),
                    sim_require_finite=True,
                    sim_require_nnan=True,
                    nc=nc,
                )
            )

        devices = jax.devices()[:n_cores]
        assert len(devices) == n_cores
        self.mesh = Mesh(np.asarray(devices), ("core",))
        self._pspec = PartitionSpec("core")
        self._fn = jax.jit(
            shard_map(
                _body,
                mesh=self.mesh,
                in_specs=(self._pspec,) * (n_params + len(out_names)),
                out_specs=(self._pspec,) * len(out_names),
                check_rep=False,
            )
        )

    def put_inputs(self, in_maps):
        import jax

        concat = [
            np.concatenate(
                [np.asarray(in_maps[c][n]) for c in range(self.n_cores)], axis=0
            )
            for n in self.in_names
        ]
        for av in self.out_avals:
            concat.append(
                np.zeros((self.n_cores * av.shape[0], *av.shape[1:]), av.dtype)
            )
        sharding = jax.sharding.NamedSharding(self.mesh, self._pspec)
        return [jax.device_put(a, sharding) for a in concat]

    def run_device(self, dev_args):
        return self._fn(*dev_args)

    def __call__(self, in_maps):
        import jax

        out_arrs = self.run_device(self.put_inputs(in_maps))
        jax.block_until_ready(out_arrs)
        return [
            {
                n: np.asarray(out_arrs[i]).reshape(
                    self.n_cores, *self.out_avals[i].shape
                )[c]
                for i, n in enumerate(self.out_names)
            }
            for c in range(self.n_cores)
        ]


def _get_runner(t_steps=T):
    key = t_steps
    if key not in _BUILT:
        nc = build(t_steps)
        _BUILT[key] = _Runner(nc, N_CORES)
    return _BUILT[key]


def make_in_maps(inputs, W_f, b_f, W_i, b_i, W_g, b_g, W_o, b_o, t_steps=T):
    W_all = np.concatenate([W_f, W_i, W_g, W_o], axis=1)  # [D+H, 4H]
    b_all = np.concatenate([b_f, b_i, b_g, b_o], axis=0)  # [4H]
    xT = np.ascontiguousarray(
        inputs[:t_steps].reshape(t_steps * B, D).T
    ).astype(np.float32)
    in_maps = []
    for c in range(N_CORES):
        cols = np.concatenate(
            [np.arange(g * H + c * HS, g * H + (c + 1) * HS) for g in range(4)]
        )
        wc = np.ascontiguousarray(W_all[:, cols]).astype(np.float32)
        bc = np.broadcast_to(b_all[cols], (128, GS)).astype(np.float32)
        in_maps.append({"xT": xT, "w": np.asarray(wc), "bias": np.asarray(bc)})
    return in_maps


def assemble(results, t_steps=T):
    stacked = np.concatenate(
        [results[c]["stacked_s"] for c in range(N_CORES)], axis=2
    )
    cx = np.concatenate([results[c]["cx_s"] for c in range(N_CORES)], axis=1)
    hx = stacked[-1]
    return stacked, hx, cx


def kernel(inputs, W_f, b_f, W_i, b_i, W_g, b_g, W_o, b_o):
    rk = _get_runner(T)
    in_maps = make_in_maps(inputs, W_f, b_f, W_i, b_i, W_g, b_g, W_o, b_o)
    results = rk(in_maps)
    return assemble(results)


# revision 10
# speedup vs baseline: 2.9235x; 2.9235x over previous
"""Trainium2 Bass kernel for a classical LSTM (T=512, B=64, D=H=1024).

Strategy (8 NeuronCores, one chip):
  - Tensor-parallel over the 4H gate dimension: core c owns a 128-column
    slice of each gate (f,i,o,g order) -> a [D+H, 512] weight slice and the
    matching H-slice of the c/h state.
  - The batch (64) is split into 2 groups of 32 whose recurrences run
    interleaved/staggered: while group A's per-step h AllGather is in
    flight, group B computes - hiding most of the exchange latency
    (independent AG chains pipeline; measured ~12us per pair vs ~20us
    for one serialized chain).
  - Per step+group: gates[32,512] = xp[t] + h_{t-1} @ W_h_slice (8
    accumulating PE matmuls, h^T stationary / weights moving), sigmoid on
    [f|i|o] block + tanh on g, DVE c/h updates, PE-transpose of the new
    h slice [32,128] -> [128,32], AllGather over 8 cores rebuilds h^T
    [1024,32] for the next step.
  - The input projection xp = x @ W_x_slice + b_slice is computed
    just-in-time ~16 steps ahead on the same cores (PE idles during the
    exchange), so it adds no wall time. X is pre-transposed and
    group-blocked on the host (pure layout marshalling).

Host <-> device contract (per core c), gate-column order [f, i, o, g],
gate g cols = [c*128:(c+1)*128] of each gate:
  inputs:  xT   [1024, 2*T*32] f32  (x transposed to [D, group, T, 32])
           w    [2048, 512]    f32  (W_all column slice, rows = [D; H])
           bias [128, 512]     f32  (bias slice broadcast to 128 rows)
  outputs: stacked_s [T, 64, 128] f32 (H-slice c of every h_t)
           cx_s      [64, 128]    f32 (H-slice c of final c)
"""
import numpy as np

N_CORES = 8
T, B, D, H = 512, 64, 1024, 1024
G = 2                      # batch groups (staggered recurrences)
BG = B // G                # 32  batch per group
HS = H // N_CORES          # 128 per-core H slice
GS = 4 * HS                # 512 per-core gate-slice width
KD = D // 128              # 8   k-tiles in D
KH = H // 128              # 8   k-tiles in H
LOOK = 16                  # phase-1 lookahead in steps (even)

_BUILT = {}


def build(t_steps=T, reps=1):
    import concourse.mybir as mybir
    import concourse.tile as tile
    from concourse import bacc
    from concourse.masks import make_identity

    f32 = mybir.dt.float32
    nc = bacc.Bacc("TRN2", target_bir_lowering=False, debug=False,
                   num_devices=N_CORES)
    xT = nc.dram_tensor("xT", [D, G * t_steps * BG], f32,
                        kind="ExternalInput").ap()
    w = nc.dram_tensor("w", [D + H, GS], f32, kind="ExternalInput").ap()
    bias = nc.dram_tensor("bias", [128, GS], f32, kind="ExternalInput").ap()
    out_st = nc.dram_tensor("stacked_s", [t_steps, B, HS], f32,
                            kind="ExternalOutput").ap()
    out_c = nc.dram_tensor("cx_s", [B, HS], f32, kind="ExternalOutput").ap()

    xT_r = xT.rearrange("(k p) m -> p k m", p=128)   # [128, KD, G*T*BG]
    w_r = w.rearrange("(k p) g -> p k g", p=128)     # [128, KD+KH, GS]

    with tile.TileContext(nc) as tc:
        with (
            tc.tile_pool(name="const", bufs=1) as constp,
            tc.tile_pool(name="state", bufs=1) as statep,
            tc.tile_pool(name="xp_ring", bufs=LOOK + 6) as xpp,
            tc.tile_pool(name="xt_in", bufs=3) as xtp,
            tc.tile_pool(name="hT", bufs=2) as hTp,
            tc.tile_pool(name="work", bufs=3) as workp,
            tc.tile_pool(name="ps_gates", bufs=2, space="PSUM") as psg,
            tc.tile_pool(name="ps_xp", bufs=2, space="PSUM") as psx,
            tc.tile_pool(name="ps_tr", bufs=2, space="PSUM") as pst,
            tc.tile_pool(name="dram", bufs=2 * G, space="DRAM") as dramp,
        ):
            w_sb = constp.tile([128, KD + KH, GS], f32, name="w_sb")
            for k in range(KD + KH):
                nc.sync.dma_start(w_sb[:, k, :], w_r[:, k, :])
            bias_sb = constp.tile([128, GS], f32, name="bias_sb")
            nc.sync.dma_start(bias_sb[:], bias[:])
            ident = constp.tile([128, 128], f32, name="ident")
            make_identity(nc, ident[:])
            c_sb = [statep.tile([BG, HS], f32, name=f"c_sb{g}")
                    for g in range(G)]

            xp_tiles = [dict() for _ in range(G)]

            def phase1_chunk(g, t0):  # noqa: E306
                """xp for group g, steps t0, t0+1 (t0 even)."""
                xt_sb = xtp.tile([128, KD, 2 * BG], f32, name="xt_sb",
                                 tag="xt_sb")
                col0 = g * t_steps * BG + t0 * BG
                nc.sync.dma_start(xt_sb[:], xT_r[:, :, col0:col0 + 2 * BG])
                ps = psx.tile([2 * BG, GS], f32, name="ps", tag="ps_xp")
                for k in range(KD):
                    nc.tensor.matmul(
                        ps[:], xt_sb[:, k, :], w_sb[:, k, :],
                        start=(k == 0), stop=(k == KD - 1),
                    )
                for j in range(2):
                    xp_t = xpp.tile([BG, GS], f32, name="xp_t", tag="xp_ring")
                    nc.vector.tensor_add(
                        xp_t[:], ps[j * BG:(j + 1) * BG, :], bias_sb[0:BG, :]
                    )
                    xp_tiles[g][t0 + j] = xp_t

            for _rep in range(reps):
              for t0 in range(0, min(LOOK, t_steps), 2):
                for g in range(G):
                    phase1_chunk(g, t0)

              hT_prev = [None] * G
              for t in range(t_steps):
                for g in range(G):
                    if t % 2 == 0 and t + LOOK < t_steps:
                        phase1_chunk(g, t + LOOK)

                    xp_t = xp_tiles[g].pop(t)
                    if t == 0:
                        gates = xp_t  # h_{-1} = 0
                    else:
                        psg_t = psg.tile([BG, GS], f32, name="psg_t",
                                         tag="psg")
                        for k in range(KH):
                            nc.tensor.matmul(
                                psg_t[:], hT_prev[g][:, k, :],
                                w_sb[:, KD + k, :],
                                start=(k == 0), stop=(k == KH - 1),
                            )
                        gates = workp.tile([BG, GS], f32, name="gates",
                                           tag="gates")
                        nc.vector.tensor_add(gates[:], psg_t[:], xp_t[:])

                    # layout [f | i | o | g_]; sigmoid(f,i,o), tanh(g_)
                    nc.scalar.activation(
                        gates[:, 0:3 * HS], gates[:, 0:3 * HS],
                        mybir.ActivationFunctionType.Sigmoid)
                    nc.scalar.activation(
                        gates[:, 3 * HS:GS], gates[:, 3 * HS:GS],
                        mybir.ActivationFunctionType.Tanh)
                    f_g = gates[:, 0:HS]
                    i_g = gates[:, HS:2 * HS]
                    o_g = gates[:, 2 * HS:3 * HS]
                    g_g = gates[:, 3 * HS:GS]

                    ig = workp.tile([BG, HS], f32, name="ig", tag="ig")
                    nc.vector.tensor_mul(ig[:], i_g, g_g)
                    if t == 0:
                        nc.vector.tensor_copy(c_sb[g][:], ig[:])
                    else:
                        fc = workp.tile([BG, HS], f32, name="fc", tag="fc")
                        nc.vector.tensor_mul(fc[:], f_g, c_sb[g][:])
                        nc.vector.tensor_add(c_sb[g][:], fc[:], ig[:])

                    th = workp.tile([BG, HS], f32, name="th", tag="th")
                    nc.scalar.activation(th[:], c_sb[g][:],
                                         mybir.ActivationFunctionType.Tanh)
                    h_sb = workp.tile([BG, HS], f32, name="h_sb", tag="h")
                    nc.vector.tensor_mul(h_sb[:], o_g, th[:])

                    nc.sync.dma_start(
                        out_st[t, g * BG:(g + 1) * BG, :], h_sb[:])

                    if t == t_steps - 1:
                        nc.sync.dma_start(
                            out_c[g * BG:(g + 1) * BG, :], c_sb[g][:])
                        continue

                    tr = pst.tile([HS, BG], f32, name="tr", tag="tr")
                    nc.tensor.transpose(tr[:], h_sb[:], ident[0:BG, 0:BG])
                    tr_sb = workp.tile([HS, BG], f32, name="tr_sb",
                                       tag="tr_sb")
                    nc.vector.tensor_copy(tr_sb[:], tr[:])
                    ag_in = dramp.tile([HS, BG], f32, name="ag_in",
                                       tag=f"ag_in{g}")
                    nc.sync.dma_start(ag_in[:], tr_sb[:])
                    ag_out = dramp.tile([H, BG], f32, name="ag_out",
                                        tag=f"ag_out{g}")
                    nc.gpsimd.collective_compute(
                        "AllGather", mybir.AluOpType.bypass,
                        ins=[ag_in.opt()], outs=[ag_out.opt()],
                        replica_groups=[list(range(N_CORES))],
                    )
                    hT = hTp.tile([128, KH, BG], f32, name="hT",
                                  tag=f"hT{g}")
                    nc.sync.dma_start(
                        hT[:], ag_out.rearrange("(k p) b -> p k b", p=128))
                    hT_prev[g] = hT

    nc.compile()
    return nc


class _Runner:
    """Reusable jitted SPMD runner (one trace/compile, many calls)."""

    def __init__(self, nc, n_cores):
        import jax
        import concourse.mybir as mybir
        from jax.sharding import Mesh, PartitionSpec
        from jax.experimental.shard_map import shard_map
        from concourse import bass2jax
        from concourse.bass2jax import _bass_exec_p, install_neuronx_cc_hook

        install_neuronx_cc_hook()
        self.nc = nc
        self.n_cores = n_cores
        partition_name = (
            nc.partition_id_tensor.name if nc.partition_id_tensor else None
        )
        in_names, out_names, out_avals = [], [], []
        for alloc in nc.m.functions[0].allocations:
            if not isinstance(alloc, mybir.MemoryLocationSet):
                continue
            name = alloc.memorylocations[0].name
            if alloc.kind == "ExternalInput":
                if name != partition_name:
                    in_names.append(name)
            elif alloc.kind == "ExternalOutput":
                out_names.append(name)
                out_avals.append(
                    jax.core.ShapedArray(
                        tuple(alloc.tensor_shape), mybir.dt.np(alloc.dtype)
                    )
                )
        self.in_names, self.out_names, self.out_avals = (
            in_names, out_names, out_avals,
        )
        n_params = len(in_names)
        all_in = list(in_names) + list(out_names)
        if partition_name is not None:
            all_in.append(partition_name)

        def _body(*args):
            operands = list(args)
            if partition_name is not None:
                operands.append(bass2jax.partition_id_tensor())
            return tuple(
                _bass_exec_p.bind(
                    *operands,
                    out_avals=tuple(out_avals),
                    in_names=tuple(all_in),
                    out_names=tuple(out_names),
                    lowering_input_output_aliases=(),
                    sim_require_finite=True,
                    sim_require_nnan=True,
                    nc=nc,
                )
            )

        devices = jax.devices()[:n_cores]
        assert len(devices) == n_cores, (
            f"need {n_cores} neuron cores, found {len(devices)}"
        )
        self.mesh = Mesh(np.asarray(devices), ("core",))
        self._pspec = PartitionSpec("core")
        self._fn = jax.jit(
            shard_map(
                _body,
                mesh=self.mesh,
                in_specs=(self._pspec,) * (n_params + len(out_names)),
                out_specs=(self._pspec,) * len(out_names),
                check_rep=False,
            )
        )

    def put_inputs(self, in_maps):
        import jax

        concat = [
            np.concatenate(
                [np.asarray(in_maps[c][n]) for c in range(self.n_cores)],
                axis=0,
            )
            for n in self.in_names
        ]
        for av in self.out_avals:
            concat.append(
                np.zeros((self.n_cores * av.shape[0], *av.shape[1:]), av.dtype)
            )
        sharding = jax.sharding.NamedSharding(self.mesh, self._pspec)
        return [jax.device_put(a, sharding) for a in concat]

    def run_device(self, dev_args):
        return self._fn(*dev_args)

    def __call__(self, in_maps):
        import jax

        out_arrs = self.run_device(self.put_inputs(in_maps))
        jax.block_until_ready(out_arrs)
        return [
            {
                n: np.asarray(out_arrs[i]).reshape(
                    self.n_cores, *self.out_avals[i].shape
                )[c]
                for i, n in enumerate(self.out_names)
            }
            for c in range(self.n_cores)
        ]


def _get_runner(t_steps=T, reps=1):
    key = (t_steps, reps)
    if key not in _BUILT:
        _BUILT[key] = _Runner(build(t_steps, reps), N_CORES)
    return _BUILT[key]


def make_in_maps(inputs, W_f, b_f, W_i, b_i, W_g, b_g, W_o, b_o, t_steps=T):
    # gate order [f, i, o, g]
    W_all = np.concatenate([W_f, W_i, W_o, W_g], axis=1)  # [D+H, 4H]
    b_all = np.concatenate([b_f, b_i, b_o, b_g], axis=0)  # [4H]
    x = np.asarray(inputs[:t_steps], dtype=np.float32)    # [T, B, D]
    # [T, G, BG, D] -> [D, G, T, BG] -> [D, G*T*BG]
    xT = np.ascontiguousarray(
        x.reshape(t_steps, G, BG, D).transpose(3, 1, 0, 2)
    ).reshape(D, G * t_steps * BG)
    in_maps = []
    for c in range(N_CORES):
        cols = np.concatenate(
            [np.arange(g * H + c * HS, g * H + (c + 1) * HS) for g in range(4)]
        )
        wc = np.ascontiguousarray(np.asarray(W_all)[:, cols], dtype=np.float32)
        bc = np.ascontiguousarray(
            np.broadcast_to(np.asarray(b_all)[cols], (128, GS))
        ).astype(np.float32)
        in_maps.append({"xT": xT, "w": wc, "bias": bc})
    return in_maps


def assemble(results, t_steps=T):
    stacked = np.concatenate(
        [results[c]["stacked_s"] for c in range(N_CORES)], axis=2
    )
    cx = np.concatenate([results[c]["cx_s"] for c in range(N_CORES)], axis=1)
    return stacked, stacked[-1], cx


def kernel(inputs, W_f, b_f, W_i, b_i, W_g, b_g, W_o, b_o):
    rk = _get_runner(T)
    in_maps = make_in_maps(inputs, W_f, b_f, W_i, b_i, W_g, b_g, W_o, b_o)
    return assemble(rk(in_maps))
